# revision 44
# baseline (speedup 1.0000x reference)
"""AdvisorCrossAttentionAdapter Trainium2 kernel (v6).

Full inputs in, full outputs out. Sharding: 8 cores = 4 batches x 2 halves.
Core 2b+j handles batch b; j indexes its 1024-row query slice and its
512-triplet share of the per-batch K/V prep. Pair exchanges (kM, vo) ride
ReduceScatter collectives carrying fp8 hi+lo planes.

v6 vs v5: every GEMM runs fp8 DoubleRow (cost model: 0.5 cycles/output-row
vs bf16's 1.0, independent of contraction depth). The former bf16 phases
(scores, out, acWo) become 2-3 fp8 plane-products built from exact hi+lo
fp8 splits: hi = f8(x), lo = f8(x - hi); ACT evicts hi straight from PSUM,
DVE (scalar_tensor_tensor) extracts lo. Numpy error model: ~0.014 vs the
0.02 gate (v5 was ~0.010).

Phase order vs the serialized collective device (15us + out_bytes/40GB/s
per op): P1 kM (3-term) -> kM RS (2MB) -> absT (hl store) -> acWo
(absT hl x wo hi) -> vo (alt hl x wvo hi + Pc x acWo hl) -> vo RS k=0
(1MB) -> vo RS k=1 (1MB) -> P3 scores (3-prod, own-t tiles first) ->
P4 out (3-prod, k0 columns first, own-t pairs first in each group).

Scales: weights x16 into fp8 normal range; kM/vo/absT carry x16, acWo's
x256 PSUM is rescaled to x16 at eviction; eT holds e/2 (exp bias -ln2) so
fp8's 240 max clears e^5.5; the 16s and the 1/2 cancel through the
row-sum reciprocal (ones = 16).
"""

import numpy as np
import ml_dtypes
from contextlib import ExitStack

P = 128
H = 2048          # hidden dim
HC = H // P       # 16 h-chunks of 128
T = 1024          # triplets per batch (advisor len 3072 / 3)
TC = T // P       # 8 t-chunks
TH = T // 2       # own t-half rows (512)
THC = TH // P     # 4 own t-chunks
S = 1024          # query rows per core (2048 / 2)
B = 4
NCORES = 8
CHC_MIN = 256     # compact rows per t-half, padded to 128 (data-keyed)
SCALE = 1.0 / float(np.sqrt(H))
LN2 = float(np.log(2.0))

bf16 = ml_dtypes.bfloat16
f8np = ml_dtypes.float8_e4m3

_compiled_nc = None


def _build_nc6(s_rows=S, t_trip=T, h=H, chc=CHC_MIN, n_dev=NCORES,
               nk=3):
    """nk: kM compute terms (3 = mt hl x a0 hl - lolo; 2 = mt hi x a0 hl)."""
    import concourse.mybir as mybir
    import concourse.tile as tile
    from concourse.tile import add_dep_helper
    from concourse import bacc

    hc = h // P
    tc_n = t_trip // P
    th = t_trip // 2
    thc = th // P
    sc_n = s_rows // P
    s512 = s_rows // 512
    n512 = h // 512
    chb = chc // P
    pm = 2 if nk == 3 else 1      # mt planes
    assert chc // P == 2, "pct DoubleRow pairing assumes chb == 2"
    assert s_rows % 512 == 0 and h % 1024 == 0 and chc % P == 0

    f32 = mybir.dt.float32
    f8 = mybir.dt.float8e4

    nc = bacc.Bacc("TRN2", target_bir_lowering=False, debug=False,
                   num_devices=n_dev)

    # DRAM I/O (fp8 planes; host pre-transposed / pre-split):
    #   mt  [hc, pm, P, h]   M.T x16 planes (o-chunk major; lhsT [i, o])
    #   a0t [P, 2, hc, th]   a0.T hi/lo (rhs for kM)
    #   wv  [hc, P, h]       Wv.T x16 hi (lhsT [i, j])
    #   sct [P, 2, hc, chc]  sc.T hi/lo (rhs for absT)
    #   wo  [2, P, hc, h]    Wo.T x16 hi/lo (rhs stream for acWo)
    #   wvo [P, hc, h]       (Wv.T @ Wo.T) x16 hi (rhs stream for vo_lin)
    #   alt [P, 2, hc, th]   adv_lin.T hi/lo (lhsT for vo_lin)
    #   pct [P, chb, th]     Pc.T (+-1 exact fp8; lhsT for scatter)
    #   hT  [P, 2, hc, s]    hidden.T own slice hi/lo (rhs for scores)
    #   msk [P, 2] f32       RS shard masks (1.0 on peer-destined shard)
    d_mt = nc.dram_tensor("mt", [hc, pm, P, h], f8, kind="ExternalInput")
    d_a0t = nc.dram_tensor("a0t", [P, 2, hc, th], f8, kind="ExternalInput")
    d_wv = nc.dram_tensor("wv", [hc, P, h], f8, kind="ExternalInput")
    d_sct = nc.dram_tensor("sct", [P, 2, hc // 2, 2, chc], f8,
                           kind="ExternalInput")
    d_wo = nc.dram_tensor("wo", [2, P, hc, h], f8, kind="ExternalInput")
    d_wvo = nc.dram_tensor("wvo", [P, hc, h], f8, kind="ExternalInput")
    d_alt = nc.dram_tensor("alt", [P, 2, hc, th], f8, kind="ExternalInput")
    d_pct = nc.dram_tensor("pct", [P, chb, th], f8, kind="ExternalInput")
    d_h = nc.dram_tensor("hT", [P, 2, hc, s_rows], f8, kind="ExternalInput")
    d_msk = nc.dram_tensor("msk", [P, 2], f32, kind="ExternalInput")
    bf = mybir.dt.bfloat16
    d_out = nc.dram_tensor("out", [s_rows, h], bf, kind="ExternalOutput")

    AF = mybir.ActivationFunctionType
    DR = mybir.MatmulPerfMode.DoubleRow
    ALU = mybir.AluOpType
    pairs = [[2 * i, 2 * i + 1] for i in range(n_dev // 2)]

    with tile.TileContext(nc) as tc, ExitStack() as ctx:
        big = ctx.enter_context(tc.tile_pool(name="big", bufs=1))
        pws = ctx.enter_context(tc.tile_pool(name="pws", bufs=5))
        pw = ctx.enter_context(tc.tile_pool(name="pw", bufs=1))
        pwo = ctx.enter_context(tc.tile_pool(name="pwo", bufs=2))
        pgs = ctx.enter_context(tc.tile_pool(name="pgs", bufs=4))
        pgo = ctx.enter_context(tc.tile_pool(name="pgo", bufs=3))
        psm = ctx.enter_context(tc.tile_pool(name="psm", bufs=1))
        pp = ctx.enter_context(tc.tile_pool(name="pp", bufs=8, space="PSUM"))
        dram = ctx.enter_context(tc.tile_pool(name="dram", bufs=1,
                                              space="DRAM"))

        # Persistent SBUF intermediates (fp8 planes). Tag chains reuse slots:
        #   tag A: a0t (P1, 16K) -> hT (P3, 32K)
        #   tag L: alt (P2, 16K) -> eT planes (P3/P4, 16K)
        kmT = big.tile([P, 2, hc, t_trip], f8, tag="K", name="kmT")
        vo = big.tile([P, 2, tc_n, h], f8, tag="V", name="vo")
        a0t_sb = big.tile([P, 2, hc, th], f8, tag="A", name="a0t_sb")
        alt_sb = big.tile([P, 2, hc, th], f8, tag="L", name="alt_sb")
        sct_sb = big.tile([P, 2, hc // 2, 2, chc], f8, tag="S",
                          name="sct_sb")
        absT_sb = big.tile([P, hc, chc], f8, tag="B", name="absT_sb")
        acWo_sb = big.tile([P, 2, chb, h], f8, tag="W", name="acWo_sb")
        pct_sb = psm.tile([P, chb, th], f8, tag="pc", name="pct_sb")
        msk_sb = psm.tile([P, 2], f32, tag="mk", name="msk_sb")
        escr = psm.tile([P, 512], f32, tag="es", name="escr")

        # Exchange buffers (internal DRAM), fp8 2-plane.
        # km_in shard s holds this core's kM planes scaled by msk[s];
        # RS(add) then delivers the peer's planes.
        km_in = dram.tile([2, 2, h, th], f8, name="km_in", uniquify=False)
        km_out = dram.tile([2, h, th], f8, name="km_out", uniquify=False)
        km_rv = km_in.rearrange("s x (oc p) t -> p oc s x t", p=P)
        vo_in = []
        vo_out = []
        vo_rv = []
        vo_w = [3 * h // 4, h // 4]     # 1536 + 512 o-columns
        for k in range(2):
            vo_in.append(dram.tile([2, 2, th, vo_w[k]], f8,
                                   name=f"vo_in{k}", uniquify=False))
            vo_out.append(dram.tile([2, th, vo_w[k]], f8,
                                    name=f"vo_out{k}", uniquify=False))
            vo_rv.append(vo_in[k].rearrange("s x (tb p) o -> p tb s x o",
                                            p=P))

        # First critical input DMAs: a0t planes (1MB each).
        nc.scalar.dma_start(a0t_sb[:, 0], d_a0t[:, 0])
        nc.gpsimd.dma_start(msk_sb[:], d_msk[:])
        nc.gpsimd.dma_start(pct_sb[:], d_pct[:])

        # ACT-written constant biases (avoid DMA'd const APs).
        zbias = psm.tile([P, 1], f32, tag="zb", name="zbias")
        nc.scalar.mul(zbias[:], msk_sb[:, 0:1], 0.0)
        lbias = psm.tile([P, 1], f32, tag="lb", name="lbias")
        nc.scalar.activation(lbias[:], zbias[:], AF.Copy, bias=-LN2)
        warm = psm.tile([P, 1], f32, tag="wm", name="warm")
        nc.scalar.copy(warm[:], zbias[:])

        # PE warm-up while the first weight tiles land (p-state ramp).
        dummy = psm.tile([P, 384], f8, tag="dm", name="dummy")
        nc.vector.memset(dummy[:], 0.0)
        for _ in range(33):
            ps_dm = pp.tile([P, 512], f32, tag="PS", name="ps_dm")
            nc.tensor.matmul(ps_dm[:, 0:384], dummy[:, 0:P], dummy[:],
                             start=True, stop=True)

        # ------------- P1: kM.T own columns = M @ a0_own.T ------------------
        # Terms (mt_plane, a0_plane): nk=3 -> (0,0)(0,1)(1,0); nk=2 ->
        # (0,0)(0,1). mt stream npre deep on scalar; wv pairs hoisted into
        # the P1 tail so absT never starves.
        npre = 5
        stream_tiles = {}
        terms_p1 = [(0, 0), (0, 1), (1, 0)] if nk == 3 else [(0, 0), (0, 1)]

        for oc in range(npre):
            mt_oc = pws.tile([P, 2, hc, P], f8, tag="W1", name="mt_oc")
            stream_tiles[oc] = mt_oc
            for pl in range(pm):
                nc.scalar.dma_start(mt_oc[:, pl], d_mt[oc, pl])
            if oc == 0:
                nc.scalar.dma_start(a0t_sb[:, 1], d_a0t[:, 1])
        wtiles = {}

        def after_stg(inst):
            add_dep_helper(inst.ins, last_stg.ins,
                           reason="bulk input DMA yields to km staging")

        def fetch_wo(key):
            # wo (hi+lo) / wvo (hi) streams in 512-wide blocks; 512B runs.
            if key in wtiles:
                return
            kind, i = key
            if kind == "wo":
                wt = pwo.tile([P, 2, hc, 512], f8, tag="W4o", name="wo_b")
                wtiles[key] = wt
                for pl in range(2):
                    di = nc.sync.dma_start(
                        wt[:, pl], d_wo[pl, :, :, i * 512:(i + 1) * 512])
                    if i < 2:
                        after_stg(di)
            else:
                if i == 3:
                    pool, tg = pwo, "W4o"
                else:
                    pool, tg = (pw, "W4") if i % 2 == 0 else (big, "S")
                wt = pool.tile([P, hc, 512], f8, tag=tg, name="wvo_b")
                wtiles[key] = wt
                di = nc.sync.dma_start(wt[:],
                                       d_wvo[:, :, i * 512:(i + 1) * 512])
                if i == 0:
                    after_stg(di)

        d_wv_p = d_wv.rearrange("c p (j q) -> p c j q", q=P)

        def fetch_wv(pq2):
            # wv in PAIRS: one 512KB DMA fills the 4KB slot.
            t_nx = pws.tile([P, 2, hc, P], f8, tag="W1", name="wv_jc")
            stream_tiles[hc + pq2] = t_nx
            nc.sync.dma_start(t_nx[:], d_wv_p[:, 2 * pq2:2 * pq2 + 2])

        for oc in range(hc):
            nx = oc + npre
            if nx < hc:
                t_nx = pws.tile([P, 2, hc, P], f8, tag="W1", name="mt_oc")
                stream_tiles[nx] = t_nx
                for pl in range(pm):
                    nc.scalar.dma_start(t_nx[:, pl], d_mt[nx, pl])
            # wv/sct deliberately NOT issued during P1: the km staging
            # DMAs must not queue behind them (DMA_ENGINES is FIFO).
            stg = pgs.tile([P, 2, 2, 512], f8, tag="VS", name="stg_k")
            mt_oc = stream_tiles.pop(oc)
            ps_k = pp.tile([P, 512], f32, tag="PS", name="ps_k")
            nt = len(terms_p1)
            for tn, (ta, tb2) in enumerate(terms_p1):
                for ii in range(hc // 2):
                    nc.tensor.matmul(
                        ps_k[:],
                        mt_oc[:, ta, 2 * ii:2 * ii + 2, :],
                        a0t_sb[:, tb2, 2 * ii:2 * ii + 2, :],
                        start=(tn == 0 and ii == 0),
                        stop=(tn == nt - 1 and ii == hc // 2 - 1),
                        perf_mode=DR)
            # hi plane straight from PSUM (ACT); lo = psum - hi (DVE).
            nc.scalar.activation(kmT[:, 0, oc, 0:th], ps_k[:], AF.Copy)
            nc.vector.scalar_tensor_tensor(
                kmT[:, 1, oc, 0:th], ps_k[:], 1.0,
                kmT[:, 0, oc, 0:th], ALU.mult, ALU.subtract)
            # Staged shards from the kmT slices: hi x msk on Pool/ACT,
            # lo x msk on DVE.
            nc.vector.tensor_scalar_mul(stg[:, 0, 0, :],
                                         kmT[:, 0, oc, 0:th],
                                         msk_sb[:, 0:1])
            nc.scalar.activation(stg[:, 1, 0, :], kmT[:, 0, oc, 0:th],
                                 AF.Copy, scale=msk_sb[:, 1:2])
            for sh in range(2):
                nc.vector.tensor_scalar_mul(stg[:, sh, 1, :],
                                            kmT[:, 1, oc, 0:th],
                                            msk_sb[:, sh:sh + 1])
            stg_dma = nc.sync.dma_start(km_rv[:, oc], stg[:])
            if oc == 6:
                last_stg = stg_dma
        nc.gpsimd.collective_compute(
            "ReduceScatter", ALU.add, replica_groups=pairs,
            ins=[km_in.opt()], outs=[km_out.opt()])
        # kM scatter-back is emitted after the first vo RS dispatch (below)
        # so its sem wait never blocks the gpsimd SEQ.

        # ------------- P2a: absT = |sc @ Wv.T|.T (hl store) -----------------
        npair = hc // 2
        after_stg(nc.sync.dma_start(sct_sb[:], d_sct[:]))
        fetch_wv(0)
        fetch_wv(1)
        for q in range(npair):
            if q + 2 < npair:
                fetch_wv(q + 2)
            if q == npair - 3:
                fetch_wo(("wo", 0))
            if q == npair - 1:
                fetch_wo(("wo", 1))
            if q == 4:
                after_stg(nc.sync.dma_start(alt_sb[:], d_alt[:]))
            wv_pair = stream_tiles.pop(hc + q)
            for half in range(2):
                jc = 2 * q + half
                ps_a = pp.tile([P, 512], f32, tag="PS", name="ps_a")
                for term in range(2):
                    for ii in range(hc // 2):
                        nc.tensor.matmul(
                            ps_a[:, 0:chc],
                            wv_pair[:, half, 2 * ii:2 * ii + 2, :],
                            sct_sb[:, term, ii],
                            start=(term == 0 and ii == 0),
                            stop=(term == 1 and ii == hc // 2 - 1),
                            perf_mode=DR)
                nc.scalar.activation(absT_sb[:, jc, :], ps_a[:, 0:chc],
                                     AF.Abs, bias=zbias[:])

        # hT planes for P3 (reuse a0t's slot; a0t dead after P1). First
        # s-half lands through acWo/vo; second s-half during P3 st=0.
        hT = big.tile([P, 2, hc, s_rows], f8, tag="A", name="hT")
        qs = max(1, hc // 2)

        # ------------- P2b: acWo = absT.T @ Wo.T (absT hl x wo hi) ----------
        # NOTE absT lo scaling: |x| = hi + lo with lo from the x16 psum, so
        # both planes share the x16 grid; acWo psum lands on x256 and is
        # rescaled to x16 at eviction.
        ho = 512
        nho = h // ho
        per_k = nho // 2
        for ot in range(nho):
            osl = slice(ot * ho, (ot + 1) * ho)
            fetch_wo(("wo", min(ot + 1, 3)))
            if ot >= 1:
                fetch_wo(("wvo", ot - 1))
            if ot in (nho - 2, nho - 1):
                pq = (ot - (nho - 2)) * qs
                nc.sync.dma_start(hT[:, 0, pq:pq + qs, 0:512],
                                  d_h[:, 0, pq:pq + qs, 0:512])
            wo_b = wtiles[("wo", ot)]
            for cb in range(chb):
                ps_c = pp.tile([P, 512], f32, tag="PS", name="ps_c")
                for pl in range(2):
                    for ii in range(hc // 2):
                        nc.tensor.matmul(
                            ps_c[:],
                            absT_sb[:, 2 * ii:2 * ii + 2,
                                    cb * P:(cb + 1) * P],
                            wo_b[:, pl, 2 * ii:2 * ii + 2, :],
                            start=(pl == 0 and ii == 0),
                            stop=(pl == 1 and ii == hc // 2 - 1),
                            perf_mode=DR)
                nc.scalar.activation(acWo_sb[:, 0, cb, osl], ps_c[:],
                                     AF.Copy, scale=1.0 / 16.0)
                nc.vector.scalar_tensor_tensor(
                    acWo_sb[:, 1, cb, osl], ps_c[:], 1.0 / 16.0,
                    acWo_sb[:, 0, cb, osl], ALU.mult, ALU.subtract)

        # ------------- P2c: vo = 16*(adv_lin @ Wvo + Pc' @ acWo) ------------
        # 2-term vo_lin (alt hl x wvo hi) + Pc x acWo planes (pct exact).
        # Staging buffered per-tb across ot pairs for 512B DMA runs.
        for ot in range(nho):
            osl = slice(ot * ho, (ot + 1) * ho)
            fetch_wo(("wvo", min(ot + 2, 3)))
            if ot in (0, 1):
                pq = ot * qs
                nc.sync.dma_start(hT[:, 1, pq:pq + qs, 0:512],
                                  d_h[:, 1, pq:pq + qs, 0:512])
            w_hi = wtiles[("wvo", ot)]
            k = 0 if ot < 3 else 1
            col = ot * ho if ot < 3 else 0
            for tb in range(thc):
                ps_v = pp.tile([P, 512], f32, tag="PS", name="ps_v")
                for ta in range(2):
                    for ii in range(hc // 2):
                        nc.tensor.matmul(
                            ps_v[:],
                            alt_sb[:, ta, 2 * ii:2 * ii + 2,
                                   tb * P:(tb + 1) * P],
                            w_hi[:, 2 * ii:2 * ii + 2, :],
                            start=(ta == 0 and ii == 0), stop=False,
                            perf_mode=DR)
                for pl in range(2):
                    nc.tensor.matmul(ps_v[:],
                                     pct_sb[:, 0:chb, tb * P:(tb + 1) * P],
                                     acWo_sb[:, pl, 0:chb, osl],
                                     start=False, stop=(pl == 1),
                                     perf_mode=DR)
                nc.scalar.activation(vo[:, 0, tb, osl], ps_v[:], AF.Copy)
                nc.vector.scalar_tensor_tensor(
                    vo[:, 1, tb, osl], ps_v[:], 1.0,
                    vo[:, 0, tb, osl], ALU.mult, ALU.subtract)
                # Stage this 512-block: hi x msk on Pool+ACT, lo on DVE.
                stg = pgs.tile([P, 2, 2, 512], f8, tag="VS", name="stg_v")
                nc.vector.tensor_scalar_mul(stg[:, 0, 0, :],
                                            vo[:, 0, tb, osl],
                                            msk_sb[:, 0:1])
                nc.scalar.activation(stg[:, 1, 0, :], vo[:, 0, tb, osl],
                                     AF.Copy, scale=msk_sb[:, 1:2])
                for sh in range(2):
                    nc.vector.tensor_scalar_mul(stg[:, sh, 1, :],
                                                vo[:, 1, tb, osl],
                                                msk_sb[:, sh:sh + 1])
                nc.sync.dma_start(vo_rv[k][:, tb, :, :, col:col + 512],
                                  stg[:])
            if ot in (2, 3):
                nc.gpsimd.collective_compute(
                    "ReduceScatter", ALU.add, replica_groups=pairs,
                    ins=[vo_in[k].opt()], outs=[vo_out[k].opt()])
        # Scatter-backs AFTER both RS dispatches: their sem waits can then
        # never head-of-line block a dispatch on the Pool SEQ.
        kmo = km_out.rearrange("x (oc p) t -> p x oc t", p=P)
        for x in range(2):
            nc.gpsimd.dma_start(kmT[:, x, :, th:t_trip], kmo[:, x])
        vo_off = [0, 3 * h // 4]
        for k in range(2):
            voo = vo_out[k].rearrange("x (tb p) o -> p x tb o", p=P)
            nblk = vo_w[k] // 512
            for blk in range(nblk):
                csl = slice(blk * 512, (blk + 1) * 512)
                dsl = slice(vo_off[k] + blk * 512,
                            vo_off[k] + (blk + 1) * 512)
                for x in range(2):
                    nc.gpsimd.dma_start(vo[:, x, thc:tc_n, dsl],
                                        voo[:, x, :, csl])

        # ------------- P3: scores^T, exp -> eT planes; sums -----------------
        # Own t-chunks (local kM) for both s-halves first, then peer chunks
        # (gated on the kM RS scatter).
        eT = big.tile([P, 2, tc_n, s_rows], f8, tag="L", name="eT")
        ones_t = psm.tile([P, 2, 1], f8, tag="o1", name="ones_t")
        nc.vector.memset(ones_t[:], 16.0)
        rcol = psm.tile([P, sc_n], f32, tag="rl", name="rcol")

        def emit_sums(st):
            # den[s] = sum_t (e/2)*16 over both planes: DR pairs (hi,lo)
            # per t-chunk, out [s-part, 1] -> reciprocal into rcol.
            for sc2 in range(4):
                sc = st * 4 + sc2
                ps_s = pp.tile([P, 512], f32, tag="PS", name="ps_s")
                for tch in range(tc_n):
                    nc.tensor.matmul(ps_s[:, 0:1],
                                     eT[:, :, tch, sc * P:(sc + 1) * P],
                                     ones_t[:],
                                     start=(tch == 0), stop=(tch == tc_n - 1),
                                     perf_mode=DR)
                nc.vector.reciprocal(rcol[:, sc:sc + 1], ps_s[:, 0:1])

        def p3_tile(st, tch):
            ps_x = pp.tile([P, 512], f32, tag="PS", name="ps_sc")
            ssl = slice(st * 512, (st + 1) * 512)
            tsl = slice(tch * P, (tch + 1) * P)
            for tn, (xk_, xh) in enumerate(((0, 0), (0, 1), (1, 0))):
                for ii in range(hc // 2):
                    nc.tensor.matmul(
                        ps_x[:],
                        kmT[:, xk_, 2 * ii:2 * ii + 2, tsl],
                        hT[:, xh, 2 * ii:2 * ii + 2, ssl],
                        start=(tn == 0 and ii == 0),
                        stop=(tn == 2 and ii == hc // 2 - 1),
                        perf_mode=DR)
            # e/2 planes: hi from ACT exp; second exp to f32 scratch, DVE
            # extracts lo.
            nc.scalar.activation(eT[:, 0, tch, ssl], ps_x[:], AF.Exp,
                                 bias=lbias[:], scale=SCALE / 16.0)
            nc.scalar.activation(escr[:], ps_x[:], AF.Exp,
                                 bias=lbias[:], scale=SCALE / 16.0)
            nc.vector.scalar_tensor_tensor(
                eT[:, 1, tch, ssl], escr[:], 1.0,
                eT[:, 0, tch, ssl], ALU.mult, ALU.subtract)

        for st in range(s512):
            for tch in range(thc):
                if st == 0 and tch < 2:
                    # Second s-half of hT (both planes), 1MB pieces.
                    pl = tch
                    nc.sync.dma_start(hT[:, pl, :, 512:1024],
                                      d_h[:, pl, :, 512:1024])
                p3_tile(st, tch)
        for st in range(s512):
            for tch in range(thc, tc_n):
                p3_tile(st, tch)
            if st == 0:
                emit_sums(0)
        emit_sums(1)
        warm2 = psm.tile([P, 1], f32, tag="w2", name="warm2")
        nc.scalar.copy(warm2[:], rcol[:, 0:1])

        # ------------- P4: out[s, o] = sum_t e[t,s] vo[t,o] * recip[s] ------
        # 3 plane-products (eh*vh, eh*vl, el*vh); k0 columns first; own-t
        # DR pairs before peer-t pairs inside each PSUM group.
        prods = [(0, 0), (0, 1), (1, 0)]

        def p4_calls(ps_o, sc, osl, half, start, stop):
            ncall = 0
            last = len(prods) * (thc // 2) - 1
            for (xe, xv) in prods:
                for q in range(thc // 2):
                    tq = half * (thc // 2) + q
                    nc.tensor.matmul(
                        ps_o[:],
                        eT[:, xe, 2 * tq:2 * tq + 2, sc * P:(sc + 1) * P],
                        vo[:, xv, 2 * tq:2 * tq + 2, osl],
                        start=(start and ncall == 0),
                        stop=(stop and ncall == last),
                        perf_mode=DR)
                    ncall += 1

        def p4_evict(ps_o, sc, osl):
            ob = pgo.tile([P, 512], bf, tag="OB", name="ob")
            nc.scalar.activation(ob[:], ps_o[:], AF.Copy,
                                 scale=rcol[:, sc:sc + 1])
            nc.sync.dma_start(d_out[sc * P:(sc + 1) * P, osl], ob[:])

        # ot3 (last 512 o-cols, peer planes arrive last): start 4 own-t
        # partial groups first so the RS wait overlaps useful work.
        osl3 = slice(3 * 512, 4 * 512)
        held = []
        for sc in range(4):
            ps_o = pp.tile([P, 512], f32, tag="PS", name="ps_o")
            p4_calls(ps_o, sc, osl3, 0, True, False)
            held.append(ps_o)
        # Keep the PE p-state warm while the first vo RS drains: one
        # PSUM tile reused serially so the pool ring never blocks.
        ps_dm2 = pp.tile([P, 512], f32, tag="PS", name="ps_dm2")
        for _ in range(90):
            nc.tensor.matmul(ps_dm2[:, 0:256], dummy[:, 0:P],
                             dummy[:, 0:256], start=True, stop=True)
        for ot in range(3):
            osl = slice(ot * 512, (ot + 1) * 512)
            for sc in range(sc_n):
                ps_o = pp.tile([P, 512], f32, tag="PS", name="ps_o")
                p4_calls(ps_o, sc, osl, 0, True, False)
                p4_calls(ps_o, sc, osl, 1, False, True)
                p4_evict(ps_o, sc, osl)
        for sc in range(4):
            p4_calls(held[sc], sc, osl3, 1, False, True)
            p4_evict(held[sc], sc, osl3)
        for sc in range(4, sc_n):
            ps_o = pp.tile([P, 512], f32, tag="PS", name="ps_o")
            p4_calls(ps_o, sc, osl3, 0, True, False)
            p4_calls(ps_o, sc, osl3, 1, False, True)
            p4_evict(ps_o, sc, osl3)

    nc.compile()
    return nc


def _to_dev_layout(x_t, rows, dtype):
    """[rows, n] fp32 -> [128, rows//128, n] contiguous in dtype."""
    rc = rows // P
    return np.ascontiguousarray(
        x_t.reshape(rc, P, -1).transpose(1, 0, 2).astype(dtype))


def _hi_lo_f8(x):
    """Exact 2-term fp8 split: x ~= hi + lo with hi = f8(x)."""
    hi = x.astype(f8np)
    lo = (x - hi.astype(np.float32)).astype(f8np)
    return hi, lo


def _planes_dev(x_t, rows):
    """[rows, n] fp32 -> [128, 2, rows//128, n] fp8 hi/lo planes."""
    hi, lo = _hi_lo_f8(np.ascontiguousarray(x_t))
    return np.ascontiguousarray(np.stack(
        [_to_dev_layout(hi.astype(np.float32), rows, f8np),
         _to_dev_layout(lo.astype(np.float32), rows, f8np)], axis=1))


def _to_chunked_flat_f8(x_t, rows):
    """[rows, n] fp32 -> [n//128, 128, rows] fp8, n-chunk major."""
    rc = rows // P
    dev = np.ascontiguousarray(
        x_t.reshape(rc, P, -1).transpose(1, 0, 2).astype(f8np))
    n = dev.shape[2]
    ch = np.ascontiguousarray(
        dev.reshape(P, rc, n // P, P).transpose(2, 0, 1, 3))
    return np.ascontiguousarray(ch.reshape(n // P, P, rc * P))


def _gate_prep_merged(trip, rid, cpad):
    """Host-side gate folding for a t-slice (see v5)."""
    t_n = trip.shape[0]
    h = trip.shape[2]
    m_and = rid == 0
    m_or = rid == 1
    m_not = rid == 2
    m_impl = rid == 3
    m_xor = rid == 4
    c0 = (rid >= 5).astype(np.float32)
    ca = m_and.astype(np.float32) - m_xor.astype(np.float32)
    cb = m_or.astype(np.float32) + m_xor.astype(np.float32)
    c1 = -(m_not.astype(np.float32))
    ci = m_impl.astype(np.float32)
    k_s = (ca + cb + c1) / 2
    k_d = (c1 - ci) / 2
    k_as = ci / 2
    k_ad = (cb - ca) / 2

    a0 = trip[:, 0]
    asum = trip[:, 1] + trip[:, 2]
    adif = trip[:, 1] - trip[:, 2]
    adv_lin = c0[:, None] * a0 + k_s[:, None] * asum + k_d[:, None] * adif

    impl_idx = np.where(m_impl)[0]
    aox_idx = np.where(m_and | m_or | m_xor)[0]
    n_i, n_a = len(impl_idx), len(aox_idx)
    assert n_i + n_a <= cpad, f"compact rows {n_i + n_a} > pad {cpad}"
    sc = np.zeros((cpad, h), np.float32)
    sc[:n_i] = k_as[impl_idx, None] * asum[impl_idx]
    sc[n_i:n_i + n_a] = np.abs(k_ad[aox_idx, None]) * adif[aox_idx]
    Pc = np.zeros((t_n, cpad), np.float32)
    Pc[impl_idx, np.arange(n_i)] = 1.0
    Pc[aox_idx, n_i + np.arange(n_a)] = np.sign(k_ad[aox_idx])
    return adv_lin, sc, Pc


def kernel(hidden_states, advisor_states, advisor_ids, Wq, Wk, Wv, Wo):
    from concourse.bass_utils import run_bass_kernel_spmd

    hs = np.asarray(hidden_states, dtype=np.float32)     # [4, 2048, 2048]
    adv = np.asarray(advisor_states, dtype=np.float32)   # [4, 3072, 2048]
    ids = np.asarray(advisor_ids)                        # [4, 3072]

    # Compact pad per t-half, sized to the data (multiple of 128).
    rid_all = ids.reshape(B, T, 3)[:, :, 0]
    need = 0
    for b in range(B):
        for j in range(2):
            r = rid_all[b, j * TH:(j + 1) * TH]
            need = max(need, int(((r == 0) | (r == 1) | (r == 3)
                                  | (r == 4)).sum()))
    chc = max(2 * P, -(-need // P) * P)

    global _compiled_nc
    if _compiled_nc is None or _compiled_nc[0] != chc:
        _compiled_nc = (chc, _build_nc6(chc=chc))
    nc = _compiled_nc[1]

    Wq = np.asarray(Wq, dtype=np.float32)
    Wk = np.asarray(Wk, dtype=np.float32)
    Wv = np.asarray(Wv, dtype=np.float32)
    Wo = np.asarray(Wo, dtype=np.float32)

    MT = Wk.T @ Wq                    # = (Wq.T @ Wk).T, [i, o]
    WvT = np.ascontiguousarray(Wv.T)  # [i, j]
    WoT = np.ascontiguousarray(Wo.T)  # [j, o]
    Wvo = WvT @ WoT                   # [i, o]

    # Weight-side planes (x16 into fp8 normal range).
    mt_hi, mt_lo = _hi_lo_f8(np.ascontiguousarray(MT) * 16.0)
    mt_dev = np.stack(
        [_to_chunked_flat_f8(mt_hi.astype(np.float32), H),
         _to_chunked_flat_f8(mt_lo.astype(np.float32), H)], axis=1)
    w_dev = {
        "mt": np.ascontiguousarray(mt_dev),          # [hc, 2, P, h]
        "wv": _to_chunked_flat_f8(WvT * 16.0, H),
        "wo": np.ascontiguousarray(np.stack(
            [_to_dev_layout(a.astype(np.float32), H, f8np)
             for a in _hi_lo_f8(WoT * 16.0)], axis=0)),  # [2, P, hc, h]
        "wvo": _to_dev_layout(Wvo * 16.0, H, f8np),  # [P, hc, h] hi only
    }

    in_maps = []
    for c in range(NCORES):
        b, j = c // 2, c % 2
        trip = adv[b].reshape(T, 3, H)[j * TH:(j + 1) * TH]
        rid = rid_all[b, j * TH:(j + 1) * TH]
        adv_lin, sc, Pc = _gate_prep_merged(trip, rid, chc)
        msk = np.zeros((P, 2), np.float32)
        msk[:, 1 - j] = 1.0
        m = {
            "a0t": _planes_dev(trip[:, 0].T, H),     # [128, 2, hc, th]
            "alt": _planes_dev(adv_lin.T, H),
            "sct": _planes_dev(sc.T, H).reshape(
                P, 2, HC // 2, 2, chc),
            "pct": _to_dev_layout(np.ascontiguousarray(Pc.T), chc, f8np),
            "hT": _planes_dev(hs[b, j * S:(j + 1) * S, :].T, H),
            "msk": msk,
            **w_dev,
        }
        in_maps.append(m)

    res = run_bass_kernel_spmd(nc, in_maps, core_ids=list(range(NCORES)))
    kernel._last_results = res

    out = np.empty((B, 2 * S, H), dtype=np.float32)
    for c in range(NCORES):
        b, j = c // 2, c % 2
        out[b, j * S:(j + 1) * S, :] = res.results[c]["out"].astype(
            np.float32)
    return out
